# revision 4
# baseline (speedup 1.0000x reference)
"""Trainium2 Bass kernel for nn_DecoupledAttentionWeight.

Computes the five projections q_sem/k_sem/q_geo/k_geo/v of x, applies RoPE to
the geo paths, the per-head sigmoid gate + per-path scaling to q (folded into
the projection weights host-side), and returns (q_cat, k_cat, vh) shaped
(B, H, T, 128) each.

Sharding over 8 NeuronCores: 2-way data-parallel over batch (batches {0,1} /
{2,3}) x 4-way tensor-parallel over heads (4 heads per core). Each core runs
one big [8192 x 2048] @ [2048 x 1536] matmul, then RoPE on the geo strips via
DVE with broadcast access patterns. Per-head output columns are packed
[q_sem|q_geo|k_sem|k_geo|v] so the sem||geo concat is free.

v3: fp8e4 DoubleRow matmul with a hi/lo 3-term split.
  y = x@w is computed as xh@wh + xl@wh + xh@wl where xh/xl (wh/wl) are the
  e4m3 value and its e4m3-quantized residual. DoubleRow contracts K=256 per
  instruction at 0.5 cycles/row, so the 3 terms cost 24 k-tiles vs bf16's 16
  -- a 1.33x streaming win (measured ~1.2e-3 rel err, well under the 2e-2
  gate). Weights are pre-scaled by 512 into fp8 range; the 1/512 unscale is
  folded into the RoPE tables and the psum->SBUF copies (tensor_scalar_mul).
  A post-Tile IR pass removes back-to-back duplicate LDWEIGHTS (bass emits
  one per matmul) so the 256-col DoubleRow weight loads stay off the
  critical path: 1 LDW per 3 psum-chunk matmuls.

Carried over from v2: per-partition-contiguous DMA layouts (8 KiB x-slab /
1.5 KiB w / 4 KiB trig / 3 KiB out runs), weight tiles split across both
HWDGE rings, k-outer matmul order so the first m-tile consumes w[k] in
arrival order during the weight load, single fused bf16 output per m-tile.
"""
import math
import os
import sys

import numpy as np

for _p in ("/opt/trn_rl_repo", os.path.expanduser("~/.axon_site/_ro/trn_rl_repo")):
    if os.path.isdir(_p) and _p not in sys.path:
        sys.path.insert(0, _p)

import ml_dtypes

import concourse.bacc as bacc
import concourse.mybir as mybir
import concourse.tile as tile
from concourse.bass_utils import run_bass_kernel_spmd

# Problem config (hardcoded from the nn.Module init)
D_MODEL = 2048
N_HEADS = 16
SEM_HD = 64
GEO_HD = 64
HEAD_DIM = 128
ROPE_DIM = 64
ROPE_HALF = ROPE_DIM // 2  # 32
ROPE_BASE = 10000.0
B, T = 4, 4096

# Sharding: 2 row groups (2 batches each) x 4 head groups (4 heads each)
N_CORES = 8
RG, HG = 2, 4
ROWS_PER_CORE = (B * T) // RG          # 8192
HEADS_PER_CORE = N_HEADS // HG         # 4
BLK = SEM_HD + GEO_HD + SEM_HD + GEO_HD + HEAD_DIM  # 384 cols per head
N_CORE = HEADS_PER_CORE * BLK          # 1536
K_TILES = D_MODEL // 128               # 16
DR_TILES = K_TILES // 2                # 8 DoubleRow k-tiles (K=256 each)
M_TILES = ROWS_PER_CORE // 128         # 64
SLAB_MT = 2                            # m_tiles per input DMA slab
SLAB_ROWS = SLAB_MT * 128              # 256
N_SLABS = M_TILES // SLAB_MT           # 32
SLAB_W = 2 * K_TILES * SLAB_ROWS       # hi/lo x 16 k x 256 m fp8 per partition
CHUNK = 512                            # psum bank / matmul moving size
N_CHUNKS = N_CORE // CHUNK             # 3
COS_SLOTS = T // 128                   # 32 distinct cos/sin row-tiles
W_SCALE = 512.0                        # weight prescale into fp8 range

_f32 = mybir.dt.float32
_bf16 = mybir.dt.bfloat16
_f8 = mybir.dt.float8e4
_np_f8 = ml_dtypes.float8_e4m3
_bf = ml_dtypes.bfloat16

# (x plane, w plane) per term: xh@wh + xl@wh + xh@wl
TERMS = [(0, 0), (1, 0), (0, 1)]


def _dedupe_ldweights(nc):
    """Remove InstLdweights that exactly repeat the previous PE LDW with only
    InstMatmult in between, merging any waits into the next matmul."""
    n_removed = 0
    for blk in nc.main_func.blocks:
        last_key = None
        keep = []
        pending_waits = []
        for inst in blk.instructions:
            nm = type(inst).__name__
            if getattr(inst, "engine", None) == mybir.EngineType.PE:
                if nm == "InstLdweights":
                    a = inst.ins[0]
                    key = (
                        a.memref,
                        a.offset,
                        tuple(tuple(p) for p in a.ap),
                        str(getattr(inst, "perf_mode", None)),
                        str(getattr(inst, "is_transpose", None)),
                        str(getattr(inst, "tile_position", None)),
                    )
                    if key == last_key:
                        si = inst.sync_info
                        if si is not None and len(si.on_wait) > 0:
                            pending_waits.extend(si.on_wait)
                        assert not (si is not None and len(si.on_update) > 0)
                        n_removed += 1
                        continue
                    last_key = key
                elif nm == "InstMatmult":
                    if pending_waits:
                        si = inst.sync_info
                        if si is None:
                            inst.sync_info = mybir.SyncInfo(
                                on_wait=list(pending_waits), on_update=[]
                            )
                        else:
                            si.on_wait = list(si.on_wait) + list(pending_waits)
                        pending_waits = []
                else:
                    last_key = None  # other PE inst may clobber weights
            keep.append(inst)
        assert not pending_waits
        blk.instructions[:] = keep
    return n_removed


def _build_nc():
    nc = bacc.Bacc("TRN2", target_bir_lowering=False, debug=False, num_devices=1)
    xs_d = nc.dram_tensor("xs", [128, N_SLABS, SLAB_W], _f8, kind="ExternalInput")
    w_d = nc.dram_tensor("w", [2, K_TILES, 128, N_CORE], _f8, kind="ExternalInput")
    cos_d = nc.dram_tensor("cos", [128, COS_SLOTS * ROPE_HALF], _f32, kind="ExternalInput")
    sin_d = nc.dram_tensor("sin", [128, COS_SLOTS * ROPE_HALF], _f32, kind="ExternalInput")
    out_d = nc.dram_tensor(
        "out", [ROWS_PER_CORE, N_CORE], _bf16, kind="ExternalOutput"
    )

    with tile.TileContext(nc) as tc:
        with (
            tc.tile_pool(name="wp", bufs=1) as wp,
            tc.tile_pool(name="xp", bufs=3) as xp,
            tc.tile_pool(name="trig", bufs=1) as trigp,
            tc.tile_pool(name="stg", bufs=3) as stgp,
            tc.tile_pool(name="tmp", bufs=2) as tmpp,
            tc.tile_pool(name="ps", bufs=2, space="PSUM") as ps,
        ):
            # cos/sin tables (pre-scaled by 1/W_SCALE host-side)
            cos_sb = trigp.tile([128, COS_SLOTS * ROPE_HALF], _f32, tag="cos")
            nc.scalar.dma_start(cos_sb[:], cos_d.ap())
            sin_sb = trigp.tile([128, COS_SLOTS * ROPE_HALF], _f32, tag="sin")
            nc.scalar.dma_start(sin_sb[:], sin_d.ap())
            cos_v = cos_sb[:].rearrange("p (s c) -> p s c", s=COS_SLOTS)
            sin_v = sin_sb[:].rearrange("p (s c) -> p s c", s=COS_SLOTS)

            slab_tiles = {}

            def load_slab(s):
                if s not in slab_tiles:
                    t = xp.tile([128, SLAB_W], _f8, tag="xt")
                    nc.scalar.dma_start(t[:], xs_d.ap()[:, s, :])
                    slab_tiles[s] = t
                return slab_tiles[s]

            # First x slab ahead of the odd-k weight loads on the scalar ring.
            load_slab(0)

            # Weight hi/lo planes resident as two [128, 16*1536] fp8 tiles.
            # Per-k slices split across both rings: even k sync, odd k scalar
            # (wh fully before wl -- the first m-tile's term order consumes
            # wh twice then wl).
            w_sb = []
            for hl in range(2):
                wt = wp.tile([128, K_TILES * N_CORE], _f8, tag=f"w{hl}")
                w_sb.append(wt)
            for hl in range(2):
                wv_t = w_sb[hl][:].rearrange("p (k n) -> p k n", k=K_TILES)
                for k in range(0, K_TILES, 2):
                    nc.sync.dma_start(wv_t[:, k, :], w_d.ap()[hl, k])
                for k in range(1, K_TILES, 2):
                    nc.scalar.dma_start(wv_t[:, k, :], w_d.ap()[hl, k])
            w_views = [
                w_sb[hl][:].rearrange("p (k n) -> p k n", k=K_TILES)
                for hl in range(2)
            ]

            inv_s = float(1.0 / W_SCALE)

            for s in range(N_SLABS):
                xt_sb = load_slab(s)
                if s + 1 < N_SLABS:
                    load_slab(s + 1)
                if s + 2 < N_SLABS:
                    load_slab(s + 2)
                xt_v = xt_sb[:].rearrange(
                    "p (h k m) -> p h k m", h=2, k=K_TILES
                )

                for i in range(SLAB_MT):
                    mt = s * SLAB_MT + i
                    psum = ps.tile([128, N_CORE], _f32, name="psum", tag="psum")
                    for ti, (xi, wi) in enumerate(TERMS):
                        for j in range(DR_TILES):
                            lhsT = xt_v[:, xi, 2 * j:2 * j + 2, i * 128:(i + 1) * 128]
                            for c in range(N_CHUNKS):
                                nc.tensor.matmul(
                                    psum[:, c * CHUNK:(c + 1) * CHUNK],
                                    lhsT,
                                    w_views[wi][:, 2 * j:2 * j + 2,
                                                c * CHUNK:(c + 1) * CHUNK],
                                    start=(ti == 0 and j == 0),
                                    stop=(ti == 2 and j == DR_TILES - 1),
                                    perf_mode=mybir.MatmulPerfMode.DoubleRow,
                                )

                    # Postprocess: RoPE on geo strips (tables carry the 1/512
                    # unscale), scaled copy of the rest.
                    # Per-head col layout: [qsem 64|qgeo 64|ksem 64|kgeo 64|v 128]
                    # viewed as (h, t3, c): t3=0 -> q(128), 1 -> k(128), 2 -> v(128)
                    pv = psum[:, :].rearrange(
                        "p (h t c) -> p h t c", h=HEADS_PER_CORE, t=3
                    )
                    stg = stgp.tile([128, N_CORE], _bf16, tag="stg")
                    sv = stg[:].rearrange(
                        "p (h t c) -> p h t c", h=HEADS_PER_CORE, t=3
                    )
                    slot = mt % COS_SLOTS
                    cos_bc = (
                        cos_v[:, slot, :]
                        .unsqueeze(1)
                        .unsqueeze(1)
                        .broadcast_to([128, HEADS_PER_CORE, 2, ROPE_HALF])
                    )
                    sin_bc = (
                        sin_v[:, slot, :]
                        .unsqueeze(1)
                        .unsqueeze(1)
                        .broadcast_to([128, HEADS_PER_CORE, 2, ROPE_HALF])
                    )
                    x1 = pv[:, :, 0:2, 64:96]
                    x2 = pv[:, :, 0:2, 96:128]
                    shp = [128, HEADS_PER_CORE, 2, ROPE_HALF]
                    t1 = tmpp.tile(shp, _f32, tag="t1")
                    t2 = tmpp.tile(shp, _f32, tag="t2")
                    t3 = tmpp.tile(shp, _f32, tag="t3")
                    t4 = tmpp.tile(shp, _f32, tag="t4")
                    nc.vector.tensor_mul(t1[:], x1, cos_bc)
                    nc.vector.tensor_mul(t2[:], x2, sin_bc)
                    nc.vector.tensor_mul(t3[:], x2, cos_bc)
                    nc.vector.tensor_mul(t4[:], x1, sin_bc)
                    nc.vector.tensor_sub(sv[:, :, 0:2, 64:96], t1[:], t2[:])
                    nc.vector.tensor_add(sv[:, :, 0:2, 96:128], t3[:], t4[:])
                    # sem halves of q and k (scaled copy)
                    nc.vector.tensor_scalar_mul(
                        sv[:, :, 0:2, 0:64], pv[:, :, 0:2, 0:64], inv_s
                    )
                    # v (scaled copy on the activation engine)
                    nc.scalar.activation(
                        sv[:, :, 2, :], pv[:, :, 2, :],
                        mybir.ActivationFunctionType.Copy, scale=inv_s,
                    )

                    m0 = mt * 128
                    nc.sync.dma_start(out_d.ap()[m0:m0 + 128, :], stg[:])

    n_rm = _dedupe_ldweights(nc)
    assert n_rm > 0
    nc.compile()
    return nc


_NC_CACHE = None
LAST_RESULTS = None


def _get_nc():
    global _NC_CACHE
    if _NC_CACHE is None:
        _NC_CACHE = _build_nc()
    return _NC_CACHE


def _host_tables(pos_offset):
    """cos/sin tables computed exactly as the reference does (f32 jax ops)."""
    import jax
    import jax.numpy as jnp

    with jax.default_device(jax.devices("cpu")[0]):
        inv_freq = ROPE_BASE ** (
            -jnp.arange(0, ROPE_HALF, dtype=jnp.float32) * (2.0 / ROPE_DIM)
        )
        pos = jnp.arange(T, dtype=jnp.float32) + jnp.float32(pos_offset)
        ang = pos[:, None] * inv_freq[None, :]
        cos = np.asarray(jnp.cos(ang), dtype=np.float32)
        sin = np.asarray(jnp.sin(ang), dtype=np.float32)
    # [T, 32] -> [p, slot*32 + c], row t = slot*128 + p; fold the 1/W_SCALE
    # psum unscale into the tables.
    cos = np.ascontiguousarray(
        cos.reshape(COS_SLOTS, 128, ROPE_HALF).transpose(1, 0, 2).reshape(128, -1)
    ) * np.float32(1.0 / W_SCALE)
    sin = np.ascontiguousarray(
        sin.reshape(COS_SLOTS, 128, ROPE_HALF).transpose(1, 0, 2).reshape(128, -1)
    ) * np.float32(1.0 / W_SCALE)
    return cos, sin


def _gate(gate_logit):
    import jax

    g = np.asarray(
        jax.nn.sigmoid(np.asarray(gate_logit, dtype=np.float32)), dtype=np.float32
    )
    return g


def kernel(x, wq_sem, wk_sem, wq_geo, wk_geo, wv, gate_logit, pos_offset):
    x = np.asarray(x, dtype=np.float32)
    wq_sem = np.asarray(wq_sem, dtype=np.float32)
    wk_sem = np.asarray(wk_sem, dtype=np.float32)
    wq_geo = np.asarray(wq_geo, dtype=np.float32)
    wk_geo = np.asarray(wk_geo, dtype=np.float32)
    wv = np.asarray(wv, dtype=np.float32)
    pos_off = int(np.asarray(pos_offset))

    g = _gate(gate_logit)  # (16,)
    sem_scale = np.float32(1.0 / math.sqrt(float(SEM_HD)))
    geo_scale = np.float32(1.0 / math.sqrt(float(GEO_HD)))
    q_sem_col = (np.float32(2.0) * g * sem_scale).astype(np.float32)   # per head
    q_geo_col = ((np.float32(2.0) - np.float32(2.0) * g) * geo_scale).astype(
        np.float32
    )

    # Per-core weight slabs, cols per head: [qsem|qgeo|ksem|kgeo|v],
    # prescaled by W_SCALE and split hi/lo into fp8, laid out
    # [hl, k, p, n] so each k-tile DMA reads 1.5 KiB/partition runs.
    w_cores = []
    for hg in range(HG):
        cols = []
        for hl in range(HEADS_PER_CORE):
            h = hg * HEADS_PER_CORE + hl
            cols.append(wq_sem[:, h * 64:(h + 1) * 64] * q_sem_col[h])
            cols.append(wq_geo[:, h * 64:(h + 1) * 64] * q_geo_col[h])
            cols.append(wk_sem[:, h * 64:(h + 1) * 64])
            cols.append(wk_geo[:, h * 64:(h + 1) * 64])
            cols.append(wv[:, h * 128:(h + 1) * 128])
        wsc = np.concatenate(cols, axis=1) * np.float32(W_SCALE)  # (2048, 1536)
        wh = wsc.astype(_np_f8)
        wl = (wsc - wh.astype(np.float32)).astype(_np_f8)
        wdev = np.empty((2, K_TILES, 128, N_CORE), _np_f8)
        wdev[0] = wh.reshape(K_TILES, 128, N_CORE)
        wdev[1] = wl.reshape(K_TILES, 128, N_CORE)
        w_cores.append(wdev)

    # x -> hi/lo fp8 planes, per-row-group slab layout [p, s, (hl, k, m)]
    xf = x.reshape(B * T, D_MODEL)
    xh = xf.astype(_np_f8)
    xl = (xf - xh.astype(np.float32)).astype(_np_f8)
    xs_rg = []
    for rg in range(RG):
        # [s, m, k, p] -> [p, s, k, m]
        ah = xh.reshape(RG, N_SLABS, SLAB_ROWS, K_TILES, 128)[rg].transpose(3, 0, 2, 1)
        al = xl.reshape(RG, N_SLABS, SLAB_ROWS, K_TILES, 128)[rg].transpose(3, 0, 2, 1)
        arr = np.stack([ah, al], axis=2)  # [p, s, hl, k, m]
        xs_rg.append(np.ascontiguousarray(arr.reshape(128, N_SLABS, SLAB_W)))

    cos, sin = _host_tables(pos_off)

    in_maps = []
    for core in range(N_CORES):
        rg, hg = core // HG, core % HG
        in_maps.append(
            {"xs": xs_rg[rg], "w": w_cores[hg], "cos": cos, "sin": sin}
        )

    nc = _get_nc()
    res = run_bass_kernel_spmd(nc, in_maps, list(range(N_CORES)))
    global LAST_RESULTS
    LAST_RESULTS = res

    q_cat = np.empty((B, N_HEADS, T, HEAD_DIM), np.float32)
    k_cat = np.empty((B, N_HEADS, T, HEAD_DIM), np.float32)
    vh = np.empty((B, N_HEADS, T, HEAD_DIM), np.float32)
    for core in range(N_CORES):
        rg, hg = core // HG, core % HG
        # (8192, 1536) bf16 -> (b_local, T, h, t3, c)
        a = np.asarray(res.results[core]["out"]).astype(np.float32)
        a = a.reshape(2, T, HEADS_PER_CORE, 3, HEAD_DIM)
        for t3_idx, dst in ((0, q_cat), (1, k_cat), (2, vh)):
            dst[
                rg * 2:(rg + 1) * 2,
                hg * HEADS_PER_CORE:(hg + 1) * HEADS_PER_CORE,
            ] = a[:, :, :, t3_idx, :].transpose(0, 2, 1, 3)
    return q_cat, k_cat, vh


# revision 6
# speedup vs baseline: 1.4864x; 1.4864x over previous
"""Trainium2 Bass kernel for nn_DecoupledAttentionWeight.

Computes the five projections q_sem/k_sem/q_geo/k_geo/v of x, applies RoPE to
the geo paths, the per-head sigmoid gate + per-path scaling to q (folded into
the projection weights host-side), and returns (q_cat, k_cat, vh) shaped
(B, H, T, 128) each.

Sharding over 8 NeuronCores: 2-way data-parallel over batch (batches {0,1} /
{2,3}) x 4-way tensor-parallel over heads (4 heads per core). Each core runs
one big [8192 x 2048] @ [2048 x 1536] matmul in bf16 (full PE speed, ~3e-3
rel err against the f32 reference) with the per-head output columns packed as
[q_sem|q_geo|k_sem|k_geo|v] so the sem||geo concat is free, then RoPE on the
geo strips via DVE with broadcast access patterns.

Structure (v4):
 - bf16 x/w/out: halves all DMA traffic vs fp32 (fp8 DoubleRow was measured
   and rejected: 1 col/cycle streaming means the precision-required 3-term
   hi/lo split costs 1.5x bf16).
 - Host-side layouts give per-partition-contiguous DMA descriptors
   (x slabs 8 KiB, w 3 KiB, cos/sin 4 KiB, out 3 KiB runs).
 - k-outer/chunk-inner matmul order: the first m-tile consumes w[k]
   incrementally as weight tiles land; stationary x-tile reused across the
   3 psum chunks.
 - Startup interleave: slab 0 split into 4 k-quarter DMAs, interleaved with
   the odd-k weight tiles on the scalar ring; even-k weights then cos/sin on
   the sync ring. First matmul starts as soon as k-quarter 0 + w[0] land.
 - One fused bf16 output DMA per m-tile; the last two m-tiles split their
   postprocess+store in half to shorten the serial tail.
"""
import math
import os
import sys

import numpy as np

for _p in ("/opt/trn_rl_repo", os.path.expanduser("~/.axon_site/_ro/trn_rl_repo")):
    if os.path.isdir(_p) and _p not in sys.path:
        sys.path.insert(0, _p)

import ml_dtypes

import concourse.bacc as bacc
import concourse.mybir as mybir
import concourse.tile as tile
from concourse.bass_utils import run_bass_kernel_spmd

# Problem config (hardcoded from the nn.Module init)
D_MODEL = 2048
N_HEADS = 16
SEM_HD = 64
GEO_HD = 64
HEAD_DIM = 128
ROPE_DIM = 64
ROPE_HALF = ROPE_DIM // 2  # 32
ROPE_BASE = 10000.0
B, T = 4, 4096

# Sharding: 2 row groups (2 batches each) x 4 head groups (4 heads each)
N_CORES = 8
RG, HG = 2, 4
ROWS_PER_CORE = (B * T) // RG          # 8192
HEADS_PER_CORE = N_HEADS // HG         # 4
BLK = SEM_HD + GEO_HD + SEM_HD + GEO_HD + HEAD_DIM  # 384 cols per head
N_CORE = HEADS_PER_CORE * BLK          # 1536
K_TILES = D_MODEL // 128               # 16
M_TILES = ROWS_PER_CORE // 128         # 64
SLAB_MT = 2                            # m_tiles per input DMA slab
SLAB_ROWS = SLAB_MT * 128              # 256
N_SLABS = M_TILES // SLAB_MT           # 32
SLAB_W = K_TILES * SLAB_ROWS           # 4096 bf16 elems per partition
CHUNK = 512                            # psum bank / matmul moving size
N_CHUNKS = N_CORE // CHUNK             # 3
COS_SLOTS = T // 128                   # 32 distinct cos/sin row-tiles

_f32 = mybir.dt.float32
_bf16 = mybir.dt.bfloat16
_bf = ml_dtypes.bfloat16


def _build_nc():
    nc = bacc.Bacc("TRN2", target_bir_lowering=False, debug=False, num_devices=1)
    xs_d = nc.dram_tensor("xs", [128, N_SLABS, SLAB_W], _bf16, kind="ExternalInput")
    w_d = nc.dram_tensor("w", [K_TILES, 128, N_CORE], _bf16, kind="ExternalInput")
    cos_d = nc.dram_tensor("cos", [128, COS_SLOTS * ROPE_HALF], _f32, kind="ExternalInput")
    sin_d = nc.dram_tensor("sin", [128, COS_SLOTS * ROPE_HALF], _f32, kind="ExternalInput")
    out_d = nc.dram_tensor(
        "out", [ROWS_PER_CORE, N_CORE], _bf16, kind="ExternalOutput"
    )

    with tile.TileContext(nc) as tc:
        with (
            tc.tile_pool(name="wp", bufs=1) as wp,
            tc.tile_pool(name="xp", bufs=3) as xp,
            tc.tile_pool(name="trig", bufs=1) as trigp,
            tc.tile_pool(name="stg", bufs=3) as stgp,
            tc.tile_pool(name="tmp", bufs=2) as tmpp,
            tc.tile_pool(name="ps", bufs=2, space="PSUM") as ps,
        ):
            slab_tiles = {}

            def load_slab(s, pieces=1):
                if s not in slab_tiles:
                    t = xp.tile([128, SLAB_W], _bf16, tag="xt")
                    step = SLAB_W // pieces
                    for j in range(pieces):
                        nc.scalar.dma_start(
                            t[:, j * step:(j + 1) * step],
                            xs_d.ap()[:, s, j * step:(j + 1) * step],
                        )
                    slab_tiles[s] = t
                return slab_tiles[s]

            w_tiles = [None] * K_TILES

            def load_w(k, ring):
                wt = wp.tile([128, N_CORE], _bf16, tag=f"w{k}")
                ring.dma_start(wt[:], w_d.ap()[k])
                w_tiles[k] = wt

            # Startup interleave. Scalar ring: slab0 k-quarter, then an odd
            # weight tile, alternating -- the first m-tile's k-loop consumes
            # both in arrival order. Sync ring: even weight tiles (w[0]
            # first), then the RoPE tables (needed ~25us in, before the
            # first output DMA is enqueued behind them).
            slab0 = xp.tile([128, SLAB_W], _bf16, tag="xt")
            q = SLAB_W // 4
            nc.scalar.dma_start(slab0[:, 0:q], xs_d.ap()[:, 0, 0:q])
            load_w(1, nc.scalar)
            nc.scalar.dma_start(slab0[:, q:2 * q], xs_d.ap()[:, 0, q:2 * q])
            load_w(3, nc.scalar)
            nc.scalar.dma_start(slab0[:, 2 * q:3 * q], xs_d.ap()[:, 0, 2 * q:3 * q])
            load_w(5, nc.scalar)
            nc.scalar.dma_start(slab0[:, 3 * q:4 * q], xs_d.ap()[:, 0, 3 * q:4 * q])
            for k in (7, 9, 11, 13, 15):
                load_w(k, nc.scalar)
            slab_tiles[0] = slab0
            for k in range(0, K_TILES, 2):
                load_w(k, nc.sync)

            cos_sb = trigp.tile([128, COS_SLOTS * ROPE_HALF], _f32, tag="cos")
            nc.sync.dma_start(cos_sb[:], cos_d.ap())
            sin_sb = trigp.tile([128, COS_SLOTS * ROPE_HALF], _f32, tag="sin")
            nc.sync.dma_start(sin_sb[:], sin_d.ap())
            cos_v = cos_sb[:].rearrange("p (s c) -> p s c", s=COS_SLOTS)
            sin_v = sin_sb[:].rearrange("p (s c) -> p s c", s=COS_SLOTS)

            def postprocess(psum, mt, h0, nh, ring):
                """RoPE + copies + output DMA for heads [h0, h0+nh) of m-tile
                mt, reading psum cols h0*BLK..(h0+nh)*BLK."""
                pv = psum[:, h0 * BLK:(h0 + nh) * BLK].rearrange(
                    "p (h t c) -> p h t c", h=nh, t=3
                )
                stg = stgp.tile([128, nh * BLK], _bf16, tag=f"stg{h0}{nh}")
                sv = stg[:].rearrange("p (h t c) -> p h t c", h=nh, t=3)
                slot = mt % COS_SLOTS
                cos_bc = (
                    cos_v[:, slot, :]
                    .unsqueeze(1)
                    .unsqueeze(1)
                    .broadcast_to([128, nh, 2, ROPE_HALF])
                )
                sin_bc = (
                    sin_v[:, slot, :]
                    .unsqueeze(1)
                    .unsqueeze(1)
                    .broadcast_to([128, nh, 2, ROPE_HALF])
                )
                x1 = pv[:, :, 0:2, 64:96]
                x2 = pv[:, :, 0:2, 96:128]
                shp = [128, nh, 2, ROPE_HALF]
                t1 = tmpp.tile(shp, _f32, tag=f"t1{h0}{nh}")
                t2 = tmpp.tile(shp, _f32, tag=f"t2{h0}{nh}")
                t3 = tmpp.tile(shp, _f32, tag=f"t3{h0}{nh}")
                t4 = tmpp.tile(shp, _f32, tag=f"t4{h0}{nh}")
                nc.vector.tensor_mul(t1[:], x1, cos_bc)
                nc.vector.tensor_mul(t2[:], x2, sin_bc)
                nc.vector.tensor_mul(t3[:], x2, cos_bc)
                nc.vector.tensor_mul(t4[:], x1, sin_bc)
                nc.vector.tensor_sub(sv[:, :, 0:2, 64:96], t1[:], t2[:])
                nc.vector.tensor_add(sv[:, :, 0:2, 96:128], t3[:], t4[:])
                # sem halves of q and k
                nc.any.tensor_copy(sv[:, :, 0:2, 0:64], pv[:, :, 0:2, 0:64])
                # v
                nc.any.tensor_copy(sv[:, :, 2, :], pv[:, :, 2, :])
                m0 = mt * 128
                ring.dma_start(
                    out_d.ap()[m0:m0 + 128, h0 * BLK:(h0 + nh) * BLK], stg[:]
                )

            for s in range(N_SLABS):
                xt_sb = load_slab(s)
                if s + 1 < N_SLABS:
                    load_slab(s + 1)
                if s + 2 < N_SLABS:
                    load_slab(s + 2)
                xt_v = xt_sb[:].rearrange("p (k m) -> p k m", k=K_TILES)

                for i in range(SLAB_MT):
                    mt = s * SLAB_MT + i
                    psum = ps.tile([128, N_CORE], _f32, name="psum", tag="psum")
                    for k in range(K_TILES):
                        for c in range(N_CHUNKS):
                            nc.tensor.matmul(
                                psum[:, c * CHUNK:(c + 1) * CHUNK],
                                xt_v[:, k, i * 128:(i + 1) * 128],
                                w_tiles[k][:, c * CHUNK:(c + 1) * CHUNK],
                                start=(k == 0),
                                stop=(k == K_TILES - 1),
                            )

                    ring = nc.sync if mt % 2 == 0 else nc.scalar
                    if mt >= M_TILES - 2:
                        # tail: split postprocess+store in half so the last
                        # DMA starts ~2us earlier
                        postprocess(psum, mt, 0, 2, ring)
                        postprocess(psum, mt, 2, 2,
                                    nc.scalar if mt % 2 == 0 else nc.sync)
                    else:
                        postprocess(psum, mt, 0, HEADS_PER_CORE, ring)

    nc.compile()
    return nc


_NC_CACHE = None
LAST_RESULTS = None


def _get_nc():
    global _NC_CACHE
    if _NC_CACHE is None:
        _NC_CACHE = _build_nc()
    return _NC_CACHE


def _host_tables(pos_offset):
    """cos/sin tables computed exactly as the reference does (f32 jax ops)."""
    import jax
    import jax.numpy as jnp

    with jax.default_device(jax.devices("cpu")[0]):
        inv_freq = ROPE_BASE ** (
            -jnp.arange(0, ROPE_HALF, dtype=jnp.float32) * (2.0 / ROPE_DIM)
        )
        pos = jnp.arange(T, dtype=jnp.float32) + jnp.float32(pos_offset)
        ang = pos[:, None] * inv_freq[None, :]
        cos = np.asarray(jnp.cos(ang), dtype=np.float32)
        sin = np.asarray(jnp.sin(ang), dtype=np.float32)
    # [T, 32] -> [p, slot*32 + c], row t = slot*128 + p
    cos = np.ascontiguousarray(
        cos.reshape(COS_SLOTS, 128, ROPE_HALF).transpose(1, 0, 2).reshape(128, -1)
    )
    sin = np.ascontiguousarray(
        sin.reshape(COS_SLOTS, 128, ROPE_HALF).transpose(1, 0, 2).reshape(128, -1)
    )
    return cos, sin


def _gate(gate_logit):
    import jax

    g = np.asarray(
        jax.nn.sigmoid(np.asarray(gate_logit, dtype=np.float32)), dtype=np.float32
    )
    return g


def kernel(x, wq_sem, wk_sem, wq_geo, wk_geo, wv, gate_logit, pos_offset):
    x = np.asarray(x, dtype=np.float32)
    wq_sem = np.asarray(wq_sem, dtype=np.float32)
    wk_sem = np.asarray(wk_sem, dtype=np.float32)
    wq_geo = np.asarray(wq_geo, dtype=np.float32)
    wk_geo = np.asarray(wk_geo, dtype=np.float32)
    wv = np.asarray(wv, dtype=np.float32)
    pos_off = int(np.asarray(pos_offset))

    g = _gate(gate_logit)  # (16,)
    sem_scale = np.float32(1.0 / math.sqrt(float(SEM_HD)))
    geo_scale = np.float32(1.0 / math.sqrt(float(GEO_HD)))
    q_sem_col = (np.float32(2.0) * g * sem_scale).astype(np.float32)   # per head
    q_geo_col = ((np.float32(2.0) - np.float32(2.0) * g) * geo_scale).astype(
        np.float32
    )

    # Per-core weight slabs, cols per head: [qsem|qgeo|ksem|kgeo|v],
    # laid out [k, p, n] so each k-tile DMA reads 3 KiB/partition runs.
    w_cores = []
    for hg in range(HG):
        cols = []
        for hl in range(HEADS_PER_CORE):
            h = hg * HEADS_PER_CORE + hl
            cols.append(wq_sem[:, h * 64:(h + 1) * 64] * q_sem_col[h])
            cols.append(wq_geo[:, h * 64:(h + 1) * 64] * q_geo_col[h])
            cols.append(wk_sem[:, h * 64:(h + 1) * 64])
            cols.append(wk_geo[:, h * 64:(h + 1) * 64])
            cols.append(wv[:, h * 128:(h + 1) * 128])
        wc = np.concatenate(cols, axis=1).astype(_bf)       # (2048, 1536)
        w_cores.append(np.ascontiguousarray(wc.reshape(K_TILES, 128, N_CORE)))

    # x -> per-row-group slab layout [p, s, k*256+m] (8 KiB contiguous
    # per partition per slab)
    xb = x.reshape(RG, N_SLABS, SLAB_ROWS, K_TILES, 128).astype(_bf)
    xs_rg = [
        np.ascontiguousarray(xb[rg].transpose(3, 0, 2, 1).reshape(128, N_SLABS, SLAB_W))
        for rg in range(RG)
    ]

    cos, sin = _host_tables(pos_off)

    in_maps = []
    for core in range(N_CORES):
        rg, hg = core // HG, core % HG
        in_maps.append(
            {"xs": xs_rg[rg], "w": w_cores[hg], "cos": cos, "sin": sin}
        )

    nc = _get_nc()
    res = run_bass_kernel_spmd(nc, in_maps, list(range(N_CORES)))
    global LAST_RESULTS
    LAST_RESULTS = res

    q_cat = np.empty((B, N_HEADS, T, HEAD_DIM), np.float32)
    k_cat = np.empty((B, N_HEADS, T, HEAD_DIM), np.float32)
    vh = np.empty((B, N_HEADS, T, HEAD_DIM), np.float32)
    for core in range(N_CORES):
        rg, hg = core // HG, core % HG
        # (8192, 1536) bf16 -> (b_local, T, h, t3, c)
        a = np.asarray(res.results[core]["out"]).astype(np.float32)
        a = a.reshape(2, T, HEADS_PER_CORE, 3, HEAD_DIM)
        for t3_idx, dst in ((0, q_cat), (1, k_cat), (2, vh)):
            dst[
                rg * 2:(rg + 1) * 2,
                hg * HEADS_PER_CORE:(hg + 1) * HEADS_PER_CORE,
            ] = a[:, :, :, t3_idx, :].transpose(0, 2, 1, 3)
    return q_cat, k_cat, vh


# revision 7
# speedup vs baseline: 1.4890x; 1.0018x over previous
"""Trainium2 Bass kernel for nn_DecoupledAttentionWeight.

Computes the five projections q_sem/k_sem/q_geo/k_geo/v of x, applies RoPE to
the geo paths, the per-head sigmoid gate + per-path scaling to q (folded into
the projection weights host-side), and returns (q_cat, k_cat, vh) shaped
(B, H, T, 128) each.

Sharding over 8 NeuronCores: 2-way data-parallel over batch (batches {0,1} /
{2,3}) x 4-way tensor-parallel over heads (4 heads per core). Each core runs
one big [8192 x 2048] @ [2048 x 1536] matmul in bf16 (full PE speed, ~3e-3
rel err against the f32 reference) with the per-head output columns packed as
[q_sem|q_geo|k_sem|k_geo|v] so the sem||geo concat is free, then RoPE on the
geo strips via DVE with broadcast access patterns.

Structure (v4):
 - bf16 x/w/out: halves all DMA traffic vs fp32 (fp8 DoubleRow was measured
   and rejected: 1 col/cycle streaming means the precision-required 3-term
   hi/lo split costs 1.5x bf16).
 - Host-side layouts give per-partition-contiguous DMA descriptors
   (x slabs 8 KiB, w 3 KiB, cos/sin 4 KiB, out 3 KiB runs).
 - k-outer/chunk-inner matmul order: the first m-tile consumes w[k]
   incrementally as weight tiles land; stationary x-tile reused across the
   3 psum chunks.
 - Startup interleave: slab 0 split into 4 k-quarter DMAs, interleaved with
   the odd-k weight tiles on the scalar ring; even-k weights then cos/sin on
   the sync ring. First matmul starts as soon as k-quarter 0 + w[0] land.
 - One fused bf16 output DMA per m-tile; the last two m-tiles split their
   postprocess+store in half to shorten the serial tail.
"""
import math
import os
import sys

import numpy as np

for _p in ("/opt/trn_rl_repo", os.path.expanduser("~/.axon_site/_ro/trn_rl_repo")):
    if os.path.isdir(_p) and _p not in sys.path:
        sys.path.insert(0, _p)

import ml_dtypes

import concourse.bacc as bacc
import concourse.mybir as mybir
import concourse.tile as tile
from concourse.bass_utils import run_bass_kernel_spmd

# Problem config (hardcoded from the nn.Module init)
D_MODEL = 2048
N_HEADS = 16
SEM_HD = 64
GEO_HD = 64
HEAD_DIM = 128
ROPE_DIM = 64
ROPE_HALF = ROPE_DIM // 2  # 32
ROPE_BASE = 10000.0
B, T = 4, 4096

# Sharding: 2 row groups (2 batches each) x 4 head groups (4 heads each)
N_CORES = 8
RG, HG = 2, 4
ROWS_PER_CORE = (B * T) // RG          # 8192
HEADS_PER_CORE = N_HEADS // HG         # 4
BLK = SEM_HD + GEO_HD + SEM_HD + GEO_HD + HEAD_DIM  # 384 cols per head
N_CORE = HEADS_PER_CORE * BLK          # 1536
K_TILES = D_MODEL // 128               # 16
M_TILES = ROWS_PER_CORE // 128         # 64
SLAB_MT = 2                            # m_tiles per input DMA slab
SLAB_ROWS = SLAB_MT * 128              # 256
N_SLABS = M_TILES // SLAB_MT           # 32
SLAB_W = K_TILES * SLAB_ROWS           # 4096 bf16 elems per partition
CHUNK = 512                            # psum bank / matmul moving size
N_CHUNKS = N_CORE // CHUNK             # 3
COS_SLOTS = T // 128                   # 32 distinct cos/sin row-tiles

_f32 = mybir.dt.float32
_bf16 = mybir.dt.bfloat16
_bf = ml_dtypes.bfloat16


def _build_nc():
    nc = bacc.Bacc("TRN2", target_bir_lowering=False, debug=False, num_devices=1)
    xs_d = nc.dram_tensor("xs", [128, N_SLABS, SLAB_W], _bf16, kind="ExternalInput")
    w_d = nc.dram_tensor("w", [K_TILES, 128, N_CORE], _bf16, kind="ExternalInput")
    cos_d = nc.dram_tensor("cos", [128, COS_SLOTS * ROPE_HALF], _f32, kind="ExternalInput")
    sin_d = nc.dram_tensor("sin", [128, COS_SLOTS * ROPE_HALF], _f32, kind="ExternalInput")
    out_d = nc.dram_tensor(
        "out", [ROWS_PER_CORE, N_CORE], _bf16, kind="ExternalOutput"
    )

    with tile.TileContext(nc) as tc:
        with (
            tc.tile_pool(name="wp", bufs=1) as wp,
            tc.tile_pool(name="xp", bufs=3) as xp,
            tc.tile_pool(name="trig", bufs=1) as trigp,
            tc.tile_pool(name="stg", bufs=3) as stgp,
            tc.tile_pool(name="tmp", bufs=2) as tmpp,
            tc.tile_pool(name="ps", bufs=2, space="PSUM") as ps,
        ):
            slab_tiles = {}

            def load_slab(s, pieces=1):
                if s not in slab_tiles:
                    t = xp.tile([128, SLAB_W], _bf16, tag="xt")
                    step = SLAB_W // pieces
                    for j in range(pieces):
                        nc.scalar.dma_start(
                            t[:, j * step:(j + 1) * step],
                            xs_d.ap()[:, s, j * step:(j + 1) * step],
                        )
                    slab_tiles[s] = t
                return slab_tiles[s]

            w_tiles = [None] * K_TILES

            def load_w(k, ring):
                wt = wp.tile([128, N_CORE], _bf16, tag=f"w{k}")
                ring.dma_start(wt[:], w_d.ap()[k])
                w_tiles[k] = wt

            # Startup interleave. Scalar ring: slab0 k-quarter, then an odd
            # weight tile, alternating -- the first m-tile's k-loop consumes
            # both in arrival order. Sync ring: even weight tiles (w[0]
            # first), then the RoPE tables (needed ~25us in, before the
            # first output DMA is enqueued behind them).
            slab0 = xp.tile([128, SLAB_W], _bf16, tag="xt")
            q = SLAB_W // 4
            nc.scalar.dma_start(slab0[:, 0:q], xs_d.ap()[:, 0, 0:q])
            load_w(1, nc.scalar)
            nc.scalar.dma_start(slab0[:, q:2 * q], xs_d.ap()[:, 0, q:2 * q])
            load_w(3, nc.scalar)
            nc.scalar.dma_start(slab0[:, 2 * q:3 * q], xs_d.ap()[:, 0, 2 * q:3 * q])
            load_w(5, nc.scalar)
            nc.scalar.dma_start(slab0[:, 3 * q:4 * q], xs_d.ap()[:, 0, 3 * q:4 * q])
            for k in (7, 9, 11, 13, 15):
                load_w(k, nc.scalar)
            slab_tiles[0] = slab0
            for k in range(0, K_TILES, 2):
                load_w(k, nc.sync)

            cos_sb = trigp.tile([128, COS_SLOTS * ROPE_HALF], _f32, tag="cos")
            nc.sync.dma_start(cos_sb[:], cos_d.ap())
            sin_sb = trigp.tile([128, COS_SLOTS * ROPE_HALF], _f32, tag="sin")
            nc.sync.dma_start(sin_sb[:], sin_d.ap())
            cos_v = cos_sb[:].rearrange("p (s c) -> p s c", s=COS_SLOTS)
            sin_v = sin_sb[:].rearrange("p (s c) -> p s c", s=COS_SLOTS)

            def postprocess(psum, mt, h0, nh, ring):
                """RoPE + copies + output DMA for heads [h0, h0+nh) of m-tile
                mt, reading psum cols h0*BLK..(h0+nh)*BLK."""
                pv = psum[:, h0 * BLK:(h0 + nh) * BLK].rearrange(
                    "p (h t c) -> p h t c", h=nh, t=3
                )
                stg = stgp.tile([128, nh * BLK], _bf16, tag=f"stg{h0}{nh}")
                sv = stg[:].rearrange("p (h t c) -> p h t c", h=nh, t=3)
                slot = mt % COS_SLOTS
                cos_bc = (
                    cos_v[:, slot, :]
                    .unsqueeze(1)
                    .unsqueeze(1)
                    .broadcast_to([128, nh, 2, ROPE_HALF])
                )
                sin_bc = (
                    sin_v[:, slot, :]
                    .unsqueeze(1)
                    .unsqueeze(1)
                    .broadcast_to([128, nh, 2, ROPE_HALF])
                )
                x1 = pv[:, :, 0:2, 64:96]
                x2 = pv[:, :, 0:2, 96:128]
                shp = [128, nh, 2, ROPE_HALF]
                t1 = tmpp.tile(shp, _f32, tag=f"t1{h0}{nh}")
                t2 = tmpp.tile(shp, _f32, tag=f"t2{h0}{nh}")
                t3 = tmpp.tile(shp, _f32, tag=f"t3{h0}{nh}")
                t4 = tmpp.tile(shp, _f32, tag=f"t4{h0}{nh}")
                nc.vector.tensor_mul(t1[:], x1, cos_bc)
                nc.vector.tensor_mul(t2[:], x2, sin_bc)
                nc.vector.tensor_mul(t3[:], x2, cos_bc)
                nc.vector.tensor_mul(t4[:], x1, sin_bc)
                nc.vector.tensor_sub(sv[:, :, 0:2, 64:96], t1[:], t2[:])
                nc.vector.tensor_add(sv[:, :, 0:2, 96:128], t3[:], t4[:])
                # sem halves of q and k
                nc.any.tensor_copy(sv[:, :, 0:2, 0:64], pv[:, :, 0:2, 0:64])
                # v
                nc.any.tensor_copy(sv[:, :, 2, :], pv[:, :, 2, :])
                m0 = mt * 128
                ring.dma_start(
                    out_d.ap()[m0:m0 + 128, h0 * BLK:(h0 + nh) * BLK], stg[:]
                )

            def mm_k(psum, xt_v, i, k):
                for c in range(N_CHUNKS):
                    nc.tensor.matmul(
                        psum[:, c * CHUNK:(c + 1) * CHUNK],
                        xt_v[:, k, i * 128:(i + 1) * 128],
                        w_tiles[k][:, c * CHUNK:(c + 1) * CHUNK],
                        start=(k == 0),
                        stop=(k == K_TILES - 1),
                    )

            for s in range(N_SLABS):
                xt_sb = load_slab(s)
                if s + 1 < N_SLABS:
                    load_slab(s + 1)
                if s + 2 < N_SLABS:
                    load_slab(s + 2)
                xt_v = xt_sb[:].rearrange("p (k m) -> p k m", k=K_TILES)

                if s == 0:
                    # Startup: interleave both m-tiles across k so each
                    # arriving weight tile feeds 6 matmuls (~its DMA time).
                    # m-tile 0 finishes its last k-tiles first so its psum
                    # drains while m-tile 1 wraps up.
                    ps_a = ps.tile([128, N_CORE], _f32, name="psum", tag="psum")
                    ps_b = ps.tile([128, N_CORE], _f32, name="psum", tag="psum")
                    for k in range(K_TILES - 2):
                        mm_k(ps_a, xt_v, 0, k)
                        mm_k(ps_b, xt_v, 1, k)
                    for k in (K_TILES - 2, K_TILES - 1):
                        mm_k(ps_a, xt_v, 0, k)
                    for k in (K_TILES - 2, K_TILES - 1):
                        mm_k(ps_b, xt_v, 1, k)
                    postprocess(ps_a, 0, 0, HEADS_PER_CORE, nc.sync)
                    postprocess(ps_b, 1, 0, HEADS_PER_CORE, nc.scalar)
                    continue

                for i in range(SLAB_MT):
                    mt = s * SLAB_MT + i
                    psum = ps.tile([128, N_CORE], _f32, name="psum", tag="psum")
                    for k in range(K_TILES):
                        mm_k(psum, xt_v, i, k)

                    ring = nc.sync if mt % 2 == 0 else nc.scalar
                    if mt >= M_TILES - 2:
                        # tail: split postprocess+store in half so the last
                        # DMA starts ~2us earlier
                        postprocess(psum, mt, 0, 2, ring)
                        postprocess(psum, mt, 2, 2,
                                    nc.scalar if mt % 2 == 0 else nc.sync)
                    else:
                        postprocess(psum, mt, 0, HEADS_PER_CORE, ring)

    nc.compile()
    return nc


_NC_CACHE = None
LAST_RESULTS = None


def _get_nc():
    global _NC_CACHE
    if _NC_CACHE is None:
        _NC_CACHE = _build_nc()
    return _NC_CACHE


def _host_tables(pos_offset):
    """cos/sin tables computed exactly as the reference does (f32 jax ops)."""
    import jax
    import jax.numpy as jnp

    with jax.default_device(jax.devices("cpu")[0]):
        inv_freq = ROPE_BASE ** (
            -jnp.arange(0, ROPE_HALF, dtype=jnp.float32) * (2.0 / ROPE_DIM)
        )
        pos = jnp.arange(T, dtype=jnp.float32) + jnp.float32(pos_offset)
        ang = pos[:, None] * inv_freq[None, :]
        cos = np.asarray(jnp.cos(ang), dtype=np.float32)
        sin = np.asarray(jnp.sin(ang), dtype=np.float32)
    # [T, 32] -> [p, slot*32 + c], row t = slot*128 + p
    cos = np.ascontiguousarray(
        cos.reshape(COS_SLOTS, 128, ROPE_HALF).transpose(1, 0, 2).reshape(128, -1)
    )
    sin = np.ascontiguousarray(
        sin.reshape(COS_SLOTS, 128, ROPE_HALF).transpose(1, 0, 2).reshape(128, -1)
    )
    return cos, sin


def _gate(gate_logit):
    import jax

    g = np.asarray(
        jax.nn.sigmoid(np.asarray(gate_logit, dtype=np.float32)), dtype=np.float32
    )
    return g


def kernel(x, wq_sem, wk_sem, wq_geo, wk_geo, wv, gate_logit, pos_offset):
    x = np.asarray(x, dtype=np.float32)
    wq_sem = np.asarray(wq_sem, dtype=np.float32)
    wk_sem = np.asarray(wk_sem, dtype=np.float32)
    wq_geo = np.asarray(wq_geo, dtype=np.float32)
    wk_geo = np.asarray(wk_geo, dtype=np.float32)
    wv = np.asarray(wv, dtype=np.float32)
    pos_off = int(np.asarray(pos_offset))

    g = _gate(gate_logit)  # (16,)
    sem_scale = np.float32(1.0 / math.sqrt(float(SEM_HD)))
    geo_scale = np.float32(1.0 / math.sqrt(float(GEO_HD)))
    q_sem_col = (np.float32(2.0) * g * sem_scale).astype(np.float32)   # per head
    q_geo_col = ((np.float32(2.0) - np.float32(2.0) * g) * geo_scale).astype(
        np.float32
    )

    # Per-core weight slabs, cols per head: [qsem|qgeo|ksem|kgeo|v],
    # laid out [k, p, n] so each k-tile DMA reads 3 KiB/partition runs.
    w_cores = []
    for hg in range(HG):
        cols = []
        for hl in range(HEADS_PER_CORE):
            h = hg * HEADS_PER_CORE + hl
            cols.append(wq_sem[:, h * 64:(h + 1) * 64] * q_sem_col[h])
            cols.append(wq_geo[:, h * 64:(h + 1) * 64] * q_geo_col[h])
            cols.append(wk_sem[:, h * 64:(h + 1) * 64])
            cols.append(wk_geo[:, h * 64:(h + 1) * 64])
            cols.append(wv[:, h * 128:(h + 1) * 128])
        wc = np.concatenate(cols, axis=1).astype(_bf)       # (2048, 1536)
        w_cores.append(np.ascontiguousarray(wc.reshape(K_TILES, 128, N_CORE)))

    # x -> per-row-group slab layout [p, s, k*256+m] (8 KiB contiguous
    # per partition per slab)
    xb = x.reshape(RG, N_SLABS, SLAB_ROWS, K_TILES, 128).astype(_bf)
    xs_rg = [
        np.ascontiguousarray(xb[rg].transpose(3, 0, 2, 1).reshape(128, N_SLABS, SLAB_W))
        for rg in range(RG)
    ]

    cos, sin = _host_tables(pos_off)

    in_maps = []
    for core in range(N_CORES):
        rg, hg = core // HG, core % HG
        in_maps.append(
            {"xs": xs_rg[rg], "w": w_cores[hg], "cos": cos, "sin": sin}
        )

    nc = _get_nc()
    res = run_bass_kernel_spmd(nc, in_maps, list(range(N_CORES)))
    global LAST_RESULTS
    LAST_RESULTS = res

    q_cat = np.empty((B, N_HEADS, T, HEAD_DIM), np.float32)
    k_cat = np.empty((B, N_HEADS, T, HEAD_DIM), np.float32)
    vh = np.empty((B, N_HEADS, T, HEAD_DIM), np.float32)
    for core in range(N_CORES):
        rg, hg = core // HG, core % HG
        # (8192, 1536) bf16 -> (b_local, T, h, t3, c)
        a = np.asarray(res.results[core]["out"]).astype(np.float32)
        a = a.reshape(2, T, HEADS_PER_CORE, 3, HEAD_DIM)
        for t3_idx, dst in ((0, q_cat), (1, k_cat), (2, vh)):
            dst[
                rg * 2:(rg + 1) * 2,
                hg * HEADS_PER_CORE:(hg + 1) * HEADS_PER_CORE,
            ] = a[:, :, :, t3_idx, :].transpose(0, 2, 1, 3)
    return q_cat, k_cat, vh


# revision 9
# speedup vs baseline: 1.4892x; 1.0001x over previous
"""Trainium2 Bass kernel for nn_DecoupledAttentionWeight.

Computes the five projections q_sem/k_sem/q_geo/k_geo/v of x, applies RoPE to
the geo paths, the per-head sigmoid gate + per-path scaling to q (folded into
the projection weights host-side), and returns (q_cat, k_cat, vh) shaped
(B, H, T, 128) each.

Sharding over 8 NeuronCores: 2-way data-parallel over batch (batches {0,1} /
{2,3}) x 4-way tensor-parallel over heads (4 heads per core). Each core runs
one big [8192 x 2048] @ [2048 x 1536] matmul in bf16 (full PE speed, ~3e-3
rel err against the f32 reference) with the per-head output columns packed as
[q_sem|q_geo|k_sem|k_geo|v] so the sem||geo concat is free, then RoPE on the
geo strips via DVE with broadcast access patterns.

Structure (v4):
 - bf16 x/w/out: halves all DMA traffic vs fp32 (fp8 DoubleRow was measured
   and rejected: 1 col/cycle streaming means the precision-required 3-term
   hi/lo split costs 1.5x bf16).
 - Host-side layouts give per-partition-contiguous DMA descriptors
   (x slabs 8 KiB, w 3 KiB, cos/sin 4 KiB, out 3 KiB runs).
 - k-outer/chunk-inner matmul order: the first m-tile consumes w[k]
   incrementally as weight tiles land; stationary x-tile reused across the
   3 psum chunks.
 - Startup interleave: slab 0 split into 4 k-quarter DMAs, interleaved with
   the odd-k weight tiles on the scalar ring; even-k weights then cos/sin on
   the sync ring. First matmul starts as soon as k-quarter 0 + w[0] land.
 - One fused bf16 output DMA per m-tile; the last two m-tiles split their
   postprocess+store in half to shorten the serial tail.
"""
import math
import os
import sys

import numpy as np

for _p in ("/opt/trn_rl_repo", os.path.expanduser("~/.axon_site/_ro/trn_rl_repo")):
    if os.path.isdir(_p) and _p not in sys.path:
        sys.path.insert(0, _p)

import ml_dtypes

import concourse.bacc as bacc
import concourse.mybir as mybir
import concourse.tile as tile
from concourse.bass_utils import run_bass_kernel_spmd

# Problem config (hardcoded from the nn.Module init)
D_MODEL = 2048
N_HEADS = 16
SEM_HD = 64
GEO_HD = 64
HEAD_DIM = 128
ROPE_DIM = 64
ROPE_HALF = ROPE_DIM // 2  # 32
ROPE_BASE = 10000.0
B, T = 4, 4096

# Sharding: 2 row groups (2 batches each) x 4 head groups (4 heads each)
N_CORES = 8
RG, HG = 2, 4
ROWS_PER_CORE = (B * T) // RG          # 8192
HEADS_PER_CORE = N_HEADS // HG         # 4
BLK = SEM_HD + GEO_HD + SEM_HD + GEO_HD + HEAD_DIM  # 384 cols per head
N_CORE = HEADS_PER_CORE * BLK          # 1536
K_TILES = D_MODEL // 128               # 16
M_TILES = ROWS_PER_CORE // 128         # 64
SLAB_MT = 2                            # m_tiles per input DMA slab
SLAB_ROWS = SLAB_MT * 128              # 256
N_SLABS = M_TILES // SLAB_MT           # 32
SLAB_W = K_TILES * SLAB_ROWS           # 4096 bf16 elems per partition
CHUNK = 512                            # psum bank / matmul moving size
N_CHUNKS = N_CORE // CHUNK             # 3
COS_SLOTS = T // 128                   # 32 distinct cos/sin row-tiles

_f32 = mybir.dt.float32
_bf16 = mybir.dt.bfloat16
_bf = ml_dtypes.bfloat16


def _build_nc():
    nc = bacc.Bacc("TRN2", target_bir_lowering=False, debug=False, num_devices=1)
    xs_d = nc.dram_tensor("xs", [128, N_SLABS, SLAB_W], _bf16, kind="ExternalInput")
    w_d = nc.dram_tensor("w", [K_TILES, 128, N_CORE], _bf16, kind="ExternalInput")
    # RoPE tables, per slot 64 wide: cs = [cos|sin], sc = [sin|cos]
    cs_d = nc.dram_tensor("cs", [128, COS_SLOTS * ROPE_DIM], _f32, kind="ExternalInput")
    sc_d = nc.dram_tensor("sc", [128, COS_SLOTS * ROPE_DIM], _f32, kind="ExternalInput")
    out_d = nc.dram_tensor(
        "out", [ROWS_PER_CORE, N_CORE], _bf16, kind="ExternalOutput"
    )

    with tile.TileContext(nc) as tc:
        with (
            tc.tile_pool(name="wp", bufs=1) as wp,
            tc.tile_pool(name="xp", bufs=3) as xp,
            tc.tile_pool(name="trig", bufs=1) as trigp,
            tc.tile_pool(name="stg", bufs=3) as stgp,
            tc.tile_pool(name="tmp", bufs=2) as tmpp,
            tc.tile_pool(name="ps", bufs=2, space="PSUM") as ps,
        ):
            slab_tiles = {}

            def load_slab(s, pieces=1):
                if s not in slab_tiles:
                    t = xp.tile([128, SLAB_W], _bf16, tag="xt")
                    step = SLAB_W // pieces
                    for j in range(pieces):
                        nc.scalar.dma_start(
                            t[:, j * step:(j + 1) * step],
                            xs_d.ap()[:, s, j * step:(j + 1) * step],
                        )
                    slab_tiles[s] = t
                return slab_tiles[s]

            w_tiles = [None] * K_TILES

            def load_w(k, ring):
                wt = wp.tile([128, N_CORE], _bf16, tag=f"w{k}")
                ring.dma_start(wt[:], w_d.ap()[k])
                w_tiles[k] = wt

            # Startup interleave. Scalar ring: slab0 k-quarter, then an odd
            # weight tile, alternating -- the first m-tile's k-loop consumes
            # both in arrival order. Sync ring: even weight tiles (w[0]
            # first), then the RoPE tables (needed ~25us in, before the
            # first output DMA is enqueued behind them).
            slab0 = xp.tile([128, SLAB_W], _bf16, tag="xt")
            q = SLAB_W // 4
            nc.scalar.dma_start(slab0[:, 0:q], xs_d.ap()[:, 0, 0:q])
            load_w(1, nc.scalar)
            nc.scalar.dma_start(slab0[:, q:2 * q], xs_d.ap()[:, 0, q:2 * q])
            load_w(3, nc.scalar)
            nc.scalar.dma_start(slab0[:, 2 * q:3 * q], xs_d.ap()[:, 0, 2 * q:3 * q])
            load_w(5, nc.scalar)
            nc.scalar.dma_start(slab0[:, 3 * q:4 * q], xs_d.ap()[:, 0, 3 * q:4 * q])
            for k in (7, 9, 11, 13, 15):
                load_w(k, nc.scalar)
            slab_tiles[0] = slab0
            for k in range(0, K_TILES, 2):
                load_w(k, nc.sync)

            cos_sb = trigp.tile([128, COS_SLOTS * ROPE_HALF], _f32, tag="cos")
            nc.sync.dma_start(cos_sb[:], cos_d.ap())
            sin_sb = trigp.tile([128, COS_SLOTS * ROPE_HALF], _f32, tag="sin")
            nc.sync.dma_start(sin_sb[:], sin_d.ap())
            cos_v = cos_sb[:].rearrange("p (s c) -> p s c", s=COS_SLOTS)
            sin_v = sin_sb[:].rearrange("p (s c) -> p s c", s=COS_SLOTS)

            def postprocess(psum, mt, h0, nh, ring):
                """RoPE + copies + output DMA for heads [h0, h0+nh) of m-tile
                mt, reading psum cols h0*BLK..(h0+nh)*BLK."""
                pv = psum[:, h0 * BLK:(h0 + nh) * BLK].rearrange(
                    "p (h t c) -> p h t c", h=nh, t=3
                )
                stg = stgp.tile([128, nh * BLK], _bf16, tag=f"stg{h0}{nh}")
                sv = stg[:].rearrange("p (h t c) -> p h t c", h=nh, t=3)
                slot = mt % COS_SLOTS
                cos_bc = (
                    cos_v[:, slot, :]
                    .unsqueeze(1)
                    .unsqueeze(1)
                    .broadcast_to([128, nh, 2, ROPE_HALF])
                )
                sin_bc = (
                    sin_v[:, slot, :]
                    .unsqueeze(1)
                    .unsqueeze(1)
                    .broadcast_to([128, nh, 2, ROPE_HALF])
                )
                x1 = pv[:, :, 0:2, 64:96]
                x2 = pv[:, :, 0:2, 96:128]
                shp = [128, nh, 2, ROPE_HALF]
                t1 = tmpp.tile(shp, _f32, tag=f"t1{h0}{nh}")
                t2 = tmpp.tile(shp, _f32, tag=f"t2{h0}{nh}")
                t3 = tmpp.tile(shp, _f32, tag=f"t3{h0}{nh}")
                t4 = tmpp.tile(shp, _f32, tag=f"t4{h0}{nh}")
                nc.vector.tensor_mul(t1[:], x1, cos_bc)
                nc.vector.tensor_mul(t2[:], x2, sin_bc)
                nc.vector.tensor_mul(t3[:], x2, cos_bc)
                nc.vector.tensor_mul(t4[:], x1, sin_bc)
                nc.vector.tensor_sub(sv[:, :, 0:2, 64:96], t1[:], t2[:])
                nc.vector.tensor_add(sv[:, :, 0:2, 96:128], t3[:], t4[:])
                # sem halves of q and k
                nc.any.tensor_copy(sv[:, :, 0:2, 0:64], pv[:, :, 0:2, 0:64])
                # v
                nc.any.tensor_copy(sv[:, :, 2, :], pv[:, :, 2, :])
                m0 = mt * 128
                ring.dma_start(
                    out_d.ap()[m0:m0 + 128, h0 * BLK:(h0 + nh) * BLK], stg[:]
                )

            def mm_k(psum, xt_v, i, k):
                for c in range(N_CHUNKS):
                    nc.tensor.matmul(
                        psum[:, c * CHUNK:(c + 1) * CHUNK],
                        xt_v[:, k, i * 128:(i + 1) * 128],
                        w_tiles[k][:, c * CHUNK:(c + 1) * CHUNK],
                        start=(k == 0),
                        stop=(k == K_TILES - 1),
                    )

            for s in range(N_SLABS):
                xt_sb = load_slab(s)
                if s + 1 < N_SLABS:
                    load_slab(s + 1)
                if s + 2 < N_SLABS:
                    load_slab(s + 2)
                xt_v = xt_sb[:].rearrange("p (k m) -> p k m", k=K_TILES)

                if s == 0:
                    # Startup: interleave both m-tiles across k so each
                    # arriving weight tile feeds 6 matmuls (~its DMA time).
                    # m-tile 0 finishes its last k-tiles first so its psum
                    # drains while m-tile 1 wraps up.
                    ps_a = ps.tile([128, N_CORE], _f32, name="psum", tag="psum")
                    ps_b = ps.tile([128, N_CORE], _f32, name="psum", tag="psum")
                    stag = 4
                    for k in range(K_TILES - stag):
                        mm_k(ps_a, xt_v, 0, k)
                        mm_k(ps_b, xt_v, 1, k)
                    for k in range(K_TILES - stag, K_TILES):
                        mm_k(ps_a, xt_v, 0, k)
                    for k in range(K_TILES - stag, K_TILES):
                        mm_k(ps_b, xt_v, 1, k)
                    postprocess(ps_a, 0, 0, HEADS_PER_CORE, nc.sync)
                    postprocess(ps_b, 1, 0, HEADS_PER_CORE, nc.scalar)
                    continue

                for i in range(SLAB_MT):
                    mt = s * SLAB_MT + i
                    psum = ps.tile([128, N_CORE], _f32, name="psum", tag="psum")
                    for k in range(K_TILES):
                        mm_k(psum, xt_v, i, k)

                    ring = nc.sync if mt % 2 == 0 else nc.scalar
                    if mt >= M_TILES - 2:
                        # tail: split postprocess+store in half so the last
                        # DMA starts ~2us earlier
                        postprocess(psum, mt, 0, 2, ring)
                        postprocess(psum, mt, 2, 2,
                                    nc.scalar if mt % 2 == 0 else nc.sync)
                    else:
                        postprocess(psum, mt, 0, HEADS_PER_CORE, ring)

    nc.compile()
    return nc


_NC_CACHE = None
LAST_RESULTS = None


def _get_nc():
    global _NC_CACHE
    if _NC_CACHE is None:
        _NC_CACHE = _build_nc()
    return _NC_CACHE


def _host_tables(pos_offset):
    """cos/sin tables computed exactly as the reference does (f32 jax ops)."""
    import jax
    import jax.numpy as jnp

    with jax.default_device(jax.devices("cpu")[0]):
        inv_freq = ROPE_BASE ** (
            -jnp.arange(0, ROPE_HALF, dtype=jnp.float32) * (2.0 / ROPE_DIM)
        )
        pos = jnp.arange(T, dtype=jnp.float32) + jnp.float32(pos_offset)
        ang = pos[:, None] * inv_freq[None, :]
        cos = np.asarray(jnp.cos(ang), dtype=np.float32)
        sin = np.asarray(jnp.sin(ang), dtype=np.float32)
    # [T, 32] -> [p, slot*32 + c], row t = slot*128 + p
    cos = np.ascontiguousarray(
        cos.reshape(COS_SLOTS, 128, ROPE_HALF).transpose(1, 0, 2).reshape(128, -1)
    )
    sin = np.ascontiguousarray(
        sin.reshape(COS_SLOTS, 128, ROPE_HALF).transpose(1, 0, 2).reshape(128, -1)
    )
    return cos, sin


def _gate(gate_logit):
    import jax

    g = np.asarray(
        jax.nn.sigmoid(np.asarray(gate_logit, dtype=np.float32)), dtype=np.float32
    )
    return g


def kernel(x, wq_sem, wk_sem, wq_geo, wk_geo, wv, gate_logit, pos_offset):
    x = np.asarray(x, dtype=np.float32)
    wq_sem = np.asarray(wq_sem, dtype=np.float32)
    wk_sem = np.asarray(wk_sem, dtype=np.float32)
    wq_geo = np.asarray(wq_geo, dtype=np.float32)
    wk_geo = np.asarray(wk_geo, dtype=np.float32)
    wv = np.asarray(wv, dtype=np.float32)
    pos_off = int(np.asarray(pos_offset))

    g = _gate(gate_logit)  # (16,)
    sem_scale = np.float32(1.0 / math.sqrt(float(SEM_HD)))
    geo_scale = np.float32(1.0 / math.sqrt(float(GEO_HD)))
    q_sem_col = (np.float32(2.0) * g * sem_scale).astype(np.float32)   # per head
    q_geo_col = ((np.float32(2.0) - np.float32(2.0) * g) * geo_scale).astype(
        np.float32
    )

    # Per-core weight slabs, cols per head: [qsem|qgeo|ksem|kgeo|v],
    # laid out [k, p, n] so each k-tile DMA reads 3 KiB/partition runs.
    w_cores = []
    for hg in range(HG):
        cols = []
        for hl in range(HEADS_PER_CORE):
            h = hg * HEADS_PER_CORE + hl
            cols.append(wq_sem[:, h * 64:(h + 1) * 64] * q_sem_col[h])
            cols.append(wq_geo[:, h * 64:(h + 1) * 64] * q_geo_col[h])
            cols.append(wk_sem[:, h * 64:(h + 1) * 64])
            cols.append(wk_geo[:, h * 64:(h + 1) * 64])
            cols.append(wv[:, h * 128:(h + 1) * 128])
        wc = np.concatenate(cols, axis=1).astype(_bf)       # (2048, 1536)
        w_cores.append(np.ascontiguousarray(wc.reshape(K_TILES, 128, N_CORE)))

    # x -> per-row-group slab layout [p, s, k*256+m] (8 KiB contiguous
    # per partition per slab)
    xb = x.reshape(RG, N_SLABS, SLAB_ROWS, K_TILES, 128).astype(_bf)
    xs_rg = [
        np.ascontiguousarray(xb[rg].transpose(3, 0, 2, 1).reshape(128, N_SLABS, SLAB_W))
        for rg in range(RG)
    ]

    cos, sin = _host_tables(pos_off)

    in_maps = []
    for core in range(N_CORES):
        rg, hg = core // HG, core % HG
        in_maps.append(
            {"xs": xs_rg[rg], "w": w_cores[hg], "cos": cos, "sin": sin}
        )

    nc = _get_nc()
    res = run_bass_kernel_spmd(nc, in_maps, list(range(N_CORES)))
    global LAST_RESULTS
    LAST_RESULTS = res

    q_cat = np.empty((B, N_HEADS, T, HEAD_DIM), np.float32)
    k_cat = np.empty((B, N_HEADS, T, HEAD_DIM), np.float32)
    vh = np.empty((B, N_HEADS, T, HEAD_DIM), np.float32)
    for core in range(N_CORES):
        rg, hg = core // HG, core % HG
        # (8192, 1536) bf16 -> (b_local, T, h, t3, c)
        a = np.asarray(res.results[core]["out"]).astype(np.float32)
        a = a.reshape(2, T, HEADS_PER_CORE, 3, HEAD_DIM)
        for t3_idx, dst in ((0, q_cat), (1, k_cat), (2, vh)):
            dst[
                rg * 2:(rg + 1) * 2,
                hg * HEADS_PER_CORE:(hg + 1) * HEADS_PER_CORE,
            ] = a[:, :, :, t3_idx, :].transpose(0, 2, 1, 3)
    return q_cat, k_cat, vh
